# revision 43
# baseline (speedup 1.0000x reference)
"""Trainium2 Bass kernel for nn_DepressionQuestionModel.

Data-parallel over the 208 conv samples (input1 batch 64 + 9*16 questions),
26 per core on 8 NeuronCores; unit-normalized features are AllGathered and
the tiny cosine/linear/softmax tail is computed redundantly on every core.
"""
import sys

sys.path.insert(0, "/opt/trn_rl_repo")

import numpy as np
import ml_dtypes

import concourse.bass as bass
import concourse.mybir as mybir
from concourse import bacc, tile, bass_utils
from concourse.masks import make_identity

N_CORES = 8
NS = 26            # samples per core (208 = 8*26)
B, SQN = 64, 144   # input batch, question count (9*16)
CI, H = 128, 768   # conv input channels / length
LA, LA2 = 382, 191  # pooled conv1 len, twice-pooled len

BF = mybir.dt.bfloat16
F32 = mybir.dt.float32

# (name, taps, src_is_a2, rhs_len, out_len, pooled_len, const_col, t2_off)
BRANCHES = [
    ("e5", 5, False, LA, 378, 189, 0, 0),
    ("e3", 5, True, LA2, 187, 93, 0, LA),
    ("e2", 2, True, LA2, 190, 95, 1, LA + LA2),
]
D5, D2, D3 = 189, 95, 93
TW = LA + 2 * LA2  # 764, combined tap-row width of all 3 branches
# per-half pack layout (bf16 [128, 52]): transposed unit-feature sections
SEC = {"e5": 0, "e5lo": 13, "e2": 26, "e3": 39}
PW = 52
GD_ADDR_SPACE = ["Shared"]  # timing harness flips to Local
DBG = [False]               # timing/debug harness can enable extra outputs

_cached = {}


def _emit(nc, tc, io):
    xs, w1t, w2t, w22t, cb = (
        io["xs"], io["w1t"], io["w2t"], io["w22t"], io["cb"],
    )
    o_dist = io["o_dist"]

    with tc.tile_pool(name="const", bufs=1) as cpool:
        W1 = cpool.tile([128, 5, 2, 128], BF)
        W2 = cpool.tile([128, 2, 5], BF)
        W22 = cpool.tile([128, 2, 2], BF)
        CB = cpool.tile([128, 2], F32)
        IDB = cpool.tile([32, 32], BF)
        make_identity(nc, IDB[:])

        # persistent activations
        A = []
        A2 = []
        apool = tc.alloc_tile_pool(name="acts", bufs=1)
        for cc in range(2):
            A.append(apool.tile([128, NS, LA], BF, name=f"A{cc}"))
            A2.append(apool.tile([128, NS, LA2], BF, name=f"A2{cc}"))

        # -------- phases 1+2 interleaved over two sample halves ----------
        # half h covers samples h*13 .. h*13+12; phase-2 of half 0 overlaps
        # phase-1 matmuls of half 1 on the non-PE engines.
        HS = NS // 2  # 13
        feats = {}  # (name, h) -> unit-feature tile [HS, d]
        dpool_ = tc.alloc_tile_pool(name="dram", bufs=1, space="DRAM")
        PKD = [dpool_.tile([128, PW], BF, name=f"PKD{h}") for h in range(2)]
        GD = [dpool_.tile([128 * N_CORES, PW], BF, name=f"GD{h}",
                          addr_space=GD_ADDR_SPACE[0]) for h in range(2)]
        with tc.tile_pool(name="xin", bufs=3) as xpool, \
             tc.tile_pool(name="ph2", bufs=1) as fpool, \
             tc.tile_pool(name="psm", bufs=1, space="PSUM") as pp:

            def load_xin(h, lc):
                sb = HS * h
                L0 = 128 * lc
                W4 = (128 if lc < 5 else 124) + 4
                xin = xpool.tile([128, HS, W4], BF, tag="xin", name="xin")
                src = xs[:, :, L0:L0 + W4].rearrange("s c l -> c s l")
                for q in range(4):
                    s0, s1 = 4 * q, min(4 * (q + 1), HS)
                    nc.sync.dma_start(xin[:, s0:s1, :],
                                      src[:, sb + s0:sb + s1, :])
                return xin

            # interleave the first input chunk with per-slice W1 loads so
            # the first matmul's deps (xin q0 + W1[k0,cc0]) land first
            sb0 = 0
            xin00 = xpool.tile([128, HS, 132], BF, tag="xin", name="xin")
            src00 = xs[:, :, 0:132].rearrange("s c l -> c s l")
            nc.sync.dma_start(xin00[:, 0:4, :], src00[:, 0:4, :])
            nc.sync.dma_start(W1[:, 0, 0, :], w1t[:, 0, 0, :])
            nc.sync.dma_start(xin00[:, 4:8, :], src00[:, 4:8, :])
            nc.sync.dma_start(W1[:, 1, 0, :], w1t[:, 1, 0, :])
            nc.sync.dma_start(xin00[:, 8:12, :], src00[:, 8:12, :])
            nc.sync.dma_start(W1[:, 2, 0, :], w1t[:, 2, 0, :])
            nc.sync.dma_start(xin00[:, 12:13, :], src00[:, 12:13, :])
            for k in range(3, 5):
                nc.sync.dma_start(W1[:, k, 0, :], w1t[:, k, 0, :])
            for k in range(5):
                nc.sync.dma_start(W1[:, k, 1, :], w1t[:, k, 1, :])
            nc.sync.dma_start(W2[:], w2t[:])
            nc.sync.dma_start(W22[:], w22t[:])
            nc.sync.dma_start(CB[:], cb[:])

            def ph1(h):
                sb = HS * h
                for lc in range(6):
                    L0 = 128 * lc
                    W = 128 if lc < 5 else 124
                    xin = xin00 if (h == 0 and lc == 0) else load_xin(h, lc)
                    for cc in range(2):
                        for g in range(4):
                            s0 = 4 * g
                            ns = min(4, HS - s0)
                            ps = pp.tile([128, 512], F32, tag="ps1", bufs=5)
                            ps3 = ps[:].rearrange(
                                "p (s l) -> p s l", s=4)[:, :ns, :W]
                            for k in range(5):
                                nc.tensor.matmul(
                                    ps3,
                                    lhsT=W1[:, k, cc, :],
                                    rhs=xin[:, s0:s0 + ns, k:k + W],
                                    start=(k == 0),
                                    stop=(k == 4),
                                )
                            ps4 = ps[:].rearrange(
                                "p (s l two) -> p s l two", s=4, two=2
                            )[:, :ns, : W // 2, :]
                            nc.vector.tensor_reduce(
                                A[cc][:, sb + s0:sb + s0 + ns,
                                      L0 // 2:L0 // 2 + W // 2],
                                ps4,
                                axis=mybir.AxisListType.X,
                                op=mybir.AluOpType.max,
                            )
                for cc in range(2):
                    av = A[cc][:, sb:sb + HS, :].rearrange(
                        "p s (l two) -> p s l two", two=2)
                    nc.vector.tensor_tensor(
                        out=A2[cc][:, sb:sb + HS, :],
                        in0=av[:, :, :, 0],
                        in1=av[:, :, :, 1],
                        op=mybir.AluOpType.max,
                    )

            def ph2mm(h):
                # conv2 tap rows, 3 samples per psum pass at bases 0/32/64
                sb = HS * h
                NG = 5  # 13 = 4*3 + 1
                PKT = fpool.tile([128, NG, TW], F32, name=f"PKT{h}",
                                 tag="PKT", bufs=2)
                T2A = fpool.tile([HS, 5, TW], F32, name=f"T2A{h}",
                                 tag="T2A", bufs=2)
                for g in range(NG):
                    ns = min(3, HS - 3 * g)
                    for name, taps, use_a2, lb, lo, dp_, ccol, off in BRANCHES:
                        wsrc = W22 if taps == 2 else W2
                        asrc = A2 if use_a2 else A
                        ps = pp.tile([128, LA], F32, tag="ps2", bufs=3)
                        for j in range(ns):
                            s = sb + 3 * g + j
                            for cc in range(2):
                                nc.tensor.matmul(
                                    ps[32 * j:32 * j + taps, :lb],
                                    lhsT=wsrc[:, cc, :],
                                    rhs=asrc[cc][:, s, :],
                                    start=(cc == 0),
                                    stop=(cc == 1),
                                )
                        nc.any.tensor_copy(
                            PKT[0:32 * (ns - 1) + taps, g, off:off + lb],
                            ps[0:32 * (ns - 1) + taps, :lb],
                        )
                    for j in range(ns):
                        sl = 3 * g + j
                        eng = nc.sync if sl % 2 == 0 else nc.gpsimd
                        eng.dma_start(
                            T2A[sl:sl + 1, :, :],
                            PKT[32 * j:32 * j + 5, g, :],
                        )
                return T2A

            def diagnorm(h, T2A):
                for name, taps, use_a2, lb, lo, dp_, ccol, off in BRANCHES:
                    ve = nc.vector
                    R = fpool.tile([HS, lo], F32, name=f"R{name}{h}", tag="R",
                                   bufs=2)
                    ve.tensor_tensor(
                        out=R[:], in0=T2A[:, 0, off:off + lo],
                        in1=T2A[:, 1, off + 1:off + 1 + lo],
                        op=mybir.AluOpType.add,
                    )
                    for k in range(2, taps):
                        ve.tensor_tensor(
                            out=R[:], in0=R[:],
                            in1=T2A[:, k, off + k:off + k + lo],
                            op=mybir.AluOpType.add,
                        )
                    rv = R[:, : 2 * dp_].rearrange(
                        "p (l two) -> p l two", two=2)
                    PL = fpool.tile([HS, dp_], F32, name=f"PL{name}{h}",
                                    tag="PL", bufs=2)
                    ve.tensor_tensor(
                        out=PL[:], in0=rv[:, :, 0], in1=rv[:, :, 1],
                        op=mybir.AluOpType.max,
                    )
                    EF = fpool.tile([HS, dp_], F32, name=f"EF{name}{h}",
                                    tag="EF", bufs=2)
                    nc.vector.tensor_scalar_add(
                        EF[:], PL[:], CB[0:HS, ccol:ccol + 1])
                    SQ = fpool.tile([HS, dp_], F32, name=f"SQ{name}{h}",
                                    tag="SQ", bufs=2)
                    N2 = fpool.tile([HS, 1], F32, name=f"N2{name}{h}",
                                    tag="N2", bufs=2)
                    nc.scalar.activation(
                        SQ[:], EF[:], mybir.ActivationFunctionType.Square,
                        accum_out=N2[:],
                    )
                    NR = fpool.tile([HS, 1], F32, name=f"NR{name}{h}",
                                    tag="NR", bufs=2)
                    nc.scalar.activation(
                        NR[:], N2[:], mybir.ActivationFunctionType.Sqrt)
                    RN = fpool.tile([HS, 1], F32, name=f"RN{name}{h}",
                                    tag="RN", bufs=2)
                    nc.vector.reciprocal(RN[:], NR[:])
                    E = fpool.tile([HS, dp_], BF, name=f"E{name}{h}")
                    nc.vector.tensor_scalar_mul(E[:], EF[:], RN[:])
                    feats[(name, h)] = E

            STALL = [fpool.tile([128, PW], BF, name=f"STALL{h}")
                     for h in range(2)]

            def transposes(h):
                secs = [
                    (feats[("e5", h)], 0, 128, SEC["e5"]),
                    (feats[("e5", h)], 128, D5 - 128, SEC["e5lo"]),
                    (feats[("e2", h)], 0, D2, SEC["e2"]),
                    (feats[("e3", h)], 0, D3, SEC["e3"]),
                ]
                for si, (E, d0, dsz, col) in enumerate(secs):
                    pt = pp.tile([128, HS], BF, tag="ps2", bufs=3)
                    nc.tensor.transpose(
                        pt[:dsz, :], E[:, d0:d0 + dsz], IDB[:HS, :HS]
                    )
                    nc.any.tensor_copy(
                        STALL[h][:dsz, col:col + HS], pt[:dsz, :])

            def gather(h):
                nc.sync.dma_start(PKD[h][:], STALL[h][:])
                nc.gpsimd.collective_compute(
                    "AllGather",
                    mybir.AluOpType.bypass,
                    replica_groups=[list(range(N_CORES))],
                    ins=[PKD[h][:]],
                    outs=[GD[h][:]],
                )

            ph1(0)
            t2a0 = ph2mm(0)
            ph1(1)
            diagnorm(0, t2a0)
            transposes(0)
            gather(0)
            t2a1 = ph2mm(1)
            diagnorm(1, t2a1)
            transposes(1)
            gather(1)

        if DBG[0]:
            nc.sync.dma_start(io["o_gd0"][:], GD[0][:])
            nc.sync.dma_start(io["o_gd1"][:], GD[1][:])
            nc.sync.dma_start(io["o_pk0"][:], PKD[0][:])
            nc.sync.dma_start(io["o_pk1"][:], PKD[1][:])
        # ---------------- phase 3: tail (identical on all cores) ------------
        _tail(nc, tc, GD, o_dist)
        dpool_.release()
        apool.release()


def _tail(nc, tc, GD, o_dist):
    # b-sample rows: half0 -> global rows {0-12, 26-38, 52-63} (38 rows),
    #                half1 -> global rows {13-25, 39-51} (26 rows)
    with tc.tile_pool(name="tail", bufs=1) as tp, \
         tc.tile_pool(name="ps3", bufs=1, space="PSUM") as pp:
        chunks = [("e5", 128, SEC["e5"]), ("e5lo", D5 - 128, SEC["e5lo"]),
                  ("e2", D2, SEC["e2"]), ("e3", D3, SEC["e3"])]
        GDs = []
        FB = {}
        for h in range(2):
            g = tp.tile([128, N_CORES, PW], BF, name=f"GDs{h}")
            nc.sync.dma_start(
                g[:], GD[h][:].rearrange("(c r) f -> r c f", c=N_CORES))
            GDs.append(g)
        for nm, rows, col in chunks:
            FB0 = tp.tile([128, 38], BF, name=f"FB0{nm}", tag=f"FB0{nm}")
            nc.any.tensor_copy(
                FB0[:rows, 0:26].rearrange("r (c s) -> r c s", c=2),
                GDs[0][:rows, 0:2, col:col + 13],
            )
            nc.any.tensor_copy(FB0[:rows, 26:38],
                               GDs[0][:rows, 2, col:col + 12])
            FB1 = tp.tile([128, 26], BF, name=f"FB1{nm}", tag=f"FB1{nm}")
            nc.any.tensor_copy(
                FB1[:rows, 0:26].rearrange("r (c s) -> r c s", c=2),
                GDs[1][:rows, 0:2, col:col + 13],
            )
            FB[(nm, 0)] = FB0
            FB[(nm, 1)] = FB1

        # cosine dots; CS[hb] psum accumulates all 4 K-chunks x 4 q-blocks
        # q-col map: (c,h,sl) -> 26c+13h+sl-64 ; view [14:144] as (c5,s26):
        # col 14+26c+t with t=sl (h=0) or t=13+sl (h=1)
        CS = [pp.tile([38, SQN], F32, name="CS0", bufs=1),
              pp.tile([26, SQN], F32, name="CS1", bufs=1)]
        nb = [38, 26]
        qblocks = [(0, 1, 0, 2, 12, 13)]
        qblocks += [(14 + 26 * c, 27 + 26 * c, 0, 3 + c, 0, 13)
                    for c in range(5)]
        qblocks += [(1, 14, 1, 2, 0, 13)]
        qblocks += [(27 + 26 * c, 40 + 26 * c, 1, 3 + c, 0, 13)
                    for c in range(5)]
        for hb in range(2):
            for q0, q1, gh, gc, s0, s1 in qblocks:
                for ci_, (nm, rows, col) in enumerate(chunks):
                    nc.tensor.matmul(
                        CS[hb][:, q0:q1], lhsT=FB[(nm, hb)][:rows, :],
                        rhs=GDs[gh][:rows, gc, col + s0:col + s1],
                        start=(ci_ == 0), stop=(ci_ == 3))

        for hb in range(2):
            DIST = tp.tile([nb[hb], 9], F32, name=f"DIST{hb}", tag="DIST",
                           bufs=2)
            nc.vector.tensor_reduce(
                DIST[:], CS[hb][:].rearrange("p (s q) -> p s q", q=16),
                axis=mybir.AxisListType.X, op=mybir.AluOpType.add,
            )
            nc.scalar.activation(DIST[:], DIST[:],
                                 mybir.ActivationFunctionType.Copy,
                                 scale=1.0 / 48.0)
            if hb == 0:
                nc.sync.dma_start(o_dist[0:13, :], DIST[0:13, :])
                nc.sync.dma_start(o_dist[26:39, :], DIST[13:26, :])
                nc.sync.dma_start(o_dist[52:64, :], DIST[26:38, :])
            else:
                nc.sync.dma_start(o_dist[13:26, :], DIST[0:13, :])
                nc.sync.dma_start(o_dist[39:52, :], DIST[13:26, :])


def _build():
    nc = bacc.Bacc("TRN2", target_bir_lowering=False, debug=False,
                   num_devices=N_CORES)
    io = {
        "xs": nc.dram_tensor("xs", [NS, CI, H], BF, kind="ExternalInput").ap(),
        "w1t": nc.dram_tensor("w1t", [128, 5, 2, 128], BF,
                              kind="ExternalInput").ap(),
        "w2t": nc.dram_tensor("w2t", [128, 2, 5], BF,
                              kind="ExternalInput").ap(),
        "w22t": nc.dram_tensor("w22t", [128, 2, 2], BF,
                               kind="ExternalInput").ap(),
        "cb": nc.dram_tensor("cb", [128, 2], F32, kind="ExternalInput").ap(),
        "o_dist": nc.dram_tensor("o_dist", [64, 9], F32,
                                 kind="ExternalOutput").ap(),
    }
    if DBG[0]:
        io["o_gd0"] = nc.dram_tensor("o_gd0", [128 * N_CORES, PW], BF,
                                     kind="ExternalOutput").ap()
        io["o_gd1"] = nc.dram_tensor("o_gd1", [128 * N_CORES, PW], BF,
                                     kind="ExternalOutput").ap()
        io["o_pk0"] = nc.dram_tensor("o_pk0", [128, PW], BF,
                                     kind="ExternalOutput").ap()
        io["o_pk1"] = nc.dram_tensor("o_pk1", [128, PW], BF,
                                     kind="ExternalOutput").ap()
    with tile.TileContext(nc) as tc:
        _emit(nc, tc, io)
    nc.compile()
    return nc


def _prep_inputs(input1, questions, labels, conv1_w, conv1_b, conv2_w, conv2_b,
                 conv2_2_w, conv2_2_b, sym_w, sym_b):
    bf = ml_dtypes.bfloat16
    xall = np.concatenate(
        [np.asarray(input1, np.float32),
         np.asarray(questions, np.float32).reshape(SQN, CI, H)], axis=0
    ).astype(bf)
    w1t = np.ascontiguousarray(
        np.asarray(conv1_w, np.float32).reshape(2, 128, CI, 5)
        .transpose(2, 3, 0, 1)
    ).astype(bf)  # [ci, k, cc, co]
    w2t = np.ascontiguousarray(
        np.asarray(conv2_w, np.float32)[0].reshape(2, 128, 5).transpose(1, 0, 2)
    ).astype(bf)  # [ci, cc, k]
    w22t = np.ascontiguousarray(
        np.asarray(conv2_2_w, np.float32)[0].reshape(2, 128, 2)
        .transpose(1, 0, 2)
    ).astype(bf)
    c5 = float((conv2_w[0].sum(axis=1) * conv1_b).sum() + conv2_b[0])
    c2 = float((conv2_2_w[0].sum(axis=1) * conv1_b).sum() + conv2_2_b[0])
    cb = np.stack([np.full(128, c5, np.float32),
                   np.full(128, c2, np.float32)], axis=1)
    common = {"w1t": w1t, "w2t": w2t, "w22t": w22t, "cb": cb}
    in_maps = []
    for c in range(N_CORES):
        m = dict(common)
        m["xs"] = np.ascontiguousarray(xall[c * NS:(c + 1) * NS])
        in_maps.append(m)
    return in_maps


def kernel(input1, questions, labels, conv1_w, conv1_b, conv2_w, conv2_b,
           conv2_2_w, conv2_2_b, sym_w, sym_b, _trace=False):
    if "nc" not in _cached:
        _cached["nc"] = _build()
    nc = _cached["nc"]
    in_maps = _prep_inputs(input1, questions, labels, conv1_w, conv1_b,
                           conv2_w, conv2_b, conv2_2_w, conv2_2_b,
                           sym_w, sym_b)
    res = bass_utils.run_bass_kernel_spmd(
        nc, in_maps, core_ids=list(range(N_CORES)), trace=_trace,
    )
    _cached["last_results"] = res
    r0 = res.results[0]
    dist = r0["o_dist"].copy()
    # tiny tail (≈1e3 flops), same formulas as the model
    logits = dist.astype(np.float64) @ np.asarray(sym_w, np.float64).T \
        + np.asarray(sym_b, np.float64)
    m = logits.max(axis=1, keepdims=True)
    e = np.exp(logits - m)
    out = e / e.sum(axis=1, keepdims=True)
    lse = np.log(np.exp(out).sum(axis=1, keepdims=True))
    logp = out - lse
    lab = np.asarray(labels).astype(np.int64)
    loss = -np.mean(logp[np.arange(out.shape[0]), lab])
    return (np.float32(loss), out.astype(np.float32), dist)


# revision 47
# speedup vs baseline: 1.0321x; 1.0321x over previous
"""Trainium2 Bass kernel for nn_DepressionQuestionModel.

Data-parallel over the 208 conv samples (input1 batch 64 + 9*16 questions),
26 per core on 8 NeuronCores; unit-normalized features are AllGathered and
the tiny cosine/linear/softmax tail is computed redundantly on every core.
"""
import sys

sys.path.insert(0, "/opt/trn_rl_repo")

import numpy as np
import ml_dtypes

import concourse.bass as bass
import concourse.mybir as mybir
from concourse import bacc, tile, bass_utils
from concourse.masks import make_identity

N_CORES = 8
NS = 26            # samples per core (208 = 8*26)
B, SQN = 64, 144   # input batch, question count (9*16)
CI, H = 128, 768   # conv input channels / length
LA, LA2 = 382, 191  # pooled conv1 len, twice-pooled len

BF = mybir.dt.bfloat16
F32 = mybir.dt.float32

# (name, taps, krow0, rhs_len, out_len, pooled_len, const_col, t2_off)
BRANCHES = [
    ("e5", 5, 0, LA, 378, 189, 0, 0),
    ("e3", 5, 0, LA2, 187, 93, 0, LA),
    ("e2", 2, 5, LA2, 190, 95, 1, LA + LA2),
]
D5, D2, D3 = 189, 95, 93
TW = LA + 2 * LA2  # 764, combined tap-row width of all 3 branches
# per-half pack layout (bf16 [128, 52]): transposed unit-feature sections
SEC = {"e5": 0, "e5lo": 13, "e2": 26, "e3": 39}
PW = 52
GD_ADDR_SPACE = ["Shared"]  # timing harness flips to Local
DBG = [False]               # timing/debug harness can enable extra outputs

_cached = {}


def _emit(nc, tc, io):
    xs, w1t, w23t, cb = (
        io["xs"], io["w1t"], io["w23t"], io["cb"],
    )
    o_dist = io["o_dist"]

    with tc.tile_pool(name="const", bufs=1) as cpool:
        W1 = cpool.tile([128, 5, 2, 128], BF)
        W23 = cpool.tile([128, 2, 7], BF)
        CB = cpool.tile([128, 2], F32)
        IDB = cpool.tile([32, 32], BF)
        make_identity(nc, IDB[:])

        # persistent activations
        A = []
        A2 = []
        apool = tc.alloc_tile_pool(name="acts", bufs=1)
        for cc in range(2):
            A.append(apool.tile([128, NS, LA], BF, name=f"A{cc}"))
            A2.append(apool.tile([128, NS, LA2], BF, name=f"A2{cc}"))

        # -------- phases 1+2 interleaved over two sample halves ----------
        # half h covers samples h*13 .. h*13+12; phase-2 of half 0 overlaps
        # phase-1 matmuls of half 1 on the non-PE engines.
        HS = NS // 2  # 13
        feats = {}  # (name, h) -> unit-feature tile [HS, d]
        dpool_ = tc.alloc_tile_pool(name="dram", bufs=1, space="DRAM")
        PKD = [dpool_.tile([128, PW], BF, name=f"PKD{h}") for h in range(2)]
        GD = [dpool_.tile([128 * N_CORES, PW], BF, name=f"GD{h}",
                          addr_space=GD_ADDR_SPACE[0]) for h in range(2)]
        with tc.tile_pool(name="xin", bufs=3) as xpool, \
             tc.tile_pool(name="ph2", bufs=1) as fpool, \
             tc.tile_pool(name="psm", bufs=1, space="PSUM") as pp:

            def load_xin(h, lc):
                sb = HS * h
                L0 = 128 * lc
                W4 = (128 if lc < 5 else 124) + 4
                xin = xpool.tile([128, HS, W4], BF, tag="xin", name="xin")
                src = xs[:, :, L0:L0 + W4].rearrange("s c l -> c s l")
                for q in range(4):
                    s0, s1 = 4 * q, min(4 * (q + 1), HS)
                    nc.sync.dma_start(xin[:, s0:s1, :],
                                      src[:, sb + s0:sb + s1, :])
                return xin

            # interleave the first input chunk with per-slice W1 loads so
            # the first matmul's deps (xin q0 + W1[k0,cc0]) land first
            sb0 = 0
            xin00 = xpool.tile([128, HS, 132], BF, tag="xin", name="xin")
            src00 = xs[:, :, 0:132].rearrange("s c l -> c s l")
            nc.sync.dma_start(xin00[:, 0:4, :], src00[:, 0:4, :])
            nc.sync.dma_start(W1[:, 0, 0, :], w1t[:, 0, 0, :])
            nc.sync.dma_start(xin00[:, 4:8, :], src00[:, 4:8, :])
            nc.sync.dma_start(W1[:, 1, 0, :], w1t[:, 1, 0, :])
            nc.sync.dma_start(xin00[:, 8:12, :], src00[:, 8:12, :])
            nc.sync.dma_start(W1[:, 2, 0, :], w1t[:, 2, 0, :])
            nc.sync.dma_start(xin00[:, 12:13, :], src00[:, 12:13, :])
            for k in range(3, 5):
                nc.sync.dma_start(W1[:, k, 0, :], w1t[:, k, 0, :])
            for k in range(5):
                nc.sync.dma_start(W1[:, k, 1, :], w1t[:, k, 1, :])
            nc.sync.dma_start(W23[:], w23t[:])
            nc.sync.dma_start(CB[:], cb[:])

            def ph1(h):
                sb = HS * h
                for lc in range(6):
                    L0 = 128 * lc
                    W = 128 if lc < 5 else 124
                    xin = xin00 if (h == 0 and lc == 0) else load_xin(h, lc)
                    for cc in range(2):
                        for g in range(4):
                            s0 = 4 * g
                            ns = min(4, HS - s0)
                            ps = pp.tile([128, 512], F32, tag="ps1", bufs=5)
                            ps3 = ps[:].rearrange(
                                "p (s l) -> p s l", s=4)[:, :ns, :W]
                            for k in range(5):
                                nc.tensor.matmul(
                                    ps3,
                                    lhsT=W1[:, k, cc, :],
                                    rhs=xin[:, s0:s0 + ns, k:k + W],
                                    start=(k == 0),
                                    stop=(k == 4),
                                )
                            ps4 = ps[:].rearrange(
                                "p (s l two) -> p s l two", s=4, two=2
                            )[:, :ns, : W // 2, :]
                            nc.vector.tensor_reduce(
                                A[cc][:, sb + s0:sb + s0 + ns,
                                      L0 // 2:L0 // 2 + W // 2],
                                ps4,
                                axis=mybir.AxisListType.X,
                                op=mybir.AluOpType.max,
                            )
                for cc in range(2):
                    av = A[cc][:, sb:sb + HS, :].rearrange(
                        "p s (l two) -> p s l two", two=2)
                    nc.vector.tensor_tensor(
                        out=A2[cc][:, sb:sb + HS, :],
                        in0=av[:, :, :, 0],
                        in1=av[:, :, :, 1],
                        op=mybir.AluOpType.max,
                    )

            def ph2mm(h):
                # conv2 tap rows, 3 samples per psum pass at bases 0/32/64
                sb = HS * h
                NG = 5  # 13 = 4*3 + 1
                PKT = fpool.tile([128, NG, TW], F32, name=f"PKT{h}",
                                 tag="PKT", bufs=2)
                T2A = fpool.tile([HS, 7, TW], F32, name=f"T2A{h}",
                                 tag="T2A", bufs=2)
                for g in range(NG):
                    ns = min(3, HS - 3 * g)
                    # pass 1: e5 taps from A -> PKT rows 32j+0..4, cols 0:382
                    ps5 = pp.tile([128, LA], F32, tag="ps2", bufs=3,
                                  name="ps5")
                    for j in range(ns):
                        s = sb + 3 * g + j
                        for cc in range(2):
                            nc.tensor.matmul(
                                ps5[32 * j:32 * j + 5, :],
                                lhsT=W23[:, cc, 0:5],
                                rhs=A[cc][:, s, :],
                                start=(cc == 0), stop=(cc == 1),
                            )
                    nc.any.tensor_copy(
                        PKT[0:32 * (ns - 1) + 5, g, 0:LA],
                        ps5[0:32 * (ns - 1) + 5, :],
                    )
                    # pass 2: e3 (rows +0..4) and e2 (rows +5..6) share the
                    # A2 rhs via a combined 7-column lhsT
                    psc = pp.tile([128, LA], F32, tag="ps2", bufs=3,
                                  name="psc")
                    for j in range(ns):
                        s = sb + 3 * g + j
                        for cc in range(2):
                            nc.tensor.matmul(
                                psc[32 * j:32 * j + 7, 0:LA2],
                                lhsT=W23[:, cc, :],
                                rhs=A2[cc][:, s, :],
                                start=(cc == 0), stop=(cc == 1),
                            )
                    nc.any.tensor_copy(
                        PKT[0:32 * (ns - 1) + 5, g, LA:LA + LA2],
                        psc[0:32 * (ns - 1) + 5, 0:LA2],
                    )
                    nc.any.tensor_copy(
                        PKT[0:32 * (ns - 1) + 7, g, LA + LA2:TW],
                        psc[0:32 * (ns - 1) + 7, 0:LA2],
                    )
                    for j in range(ns):
                        sl = 3 * g + j
                        eng = nc.sync if sl % 2 == 0 else nc.gpsimd
                        eng.dma_start(
                            T2A[sl:sl + 1, :, :],
                            PKT[32 * j:32 * j + 7, g, :],
                        )
                return T2A

            def diagnorm(h, T2A):
                for name, taps, k0, lb, lo, dp_, ccol, off in BRANCHES:
                    ve = nc.vector
                    R = fpool.tile([HS, lo], F32, name=f"R{name}{h}", tag="R",
                                   bufs=2)
                    ve.tensor_tensor(
                        out=R[:], in0=T2A[:, k0, off:off + lo],
                        in1=T2A[:, k0 + 1, off + 1:off + 1 + lo],
                        op=mybir.AluOpType.add,
                    )
                    for k in range(2, taps):
                        ve.tensor_tensor(
                            out=R[:], in0=R[:],
                            in1=T2A[:, k0 + k, off + k:off + k + lo],
                            op=mybir.AluOpType.add,
                        )
                    rv = R[:, : 2 * dp_].rearrange(
                        "p (l two) -> p l two", two=2)
                    PL = fpool.tile([HS, dp_], F32, name=f"PL{name}{h}",
                                    tag="PL", bufs=2)
                    ve.tensor_tensor(
                        out=PL[:], in0=rv[:, :, 0], in1=rv[:, :, 1],
                        op=mybir.AluOpType.max,
                    )
                    EF = fpool.tile([HS, dp_], F32, name=f"EF{name}{h}",
                                    tag="EF", bufs=2)
                    nc.vector.tensor_scalar_add(
                        EF[:], PL[:], CB[0:HS, ccol:ccol + 1])
                    SQ = fpool.tile([HS, dp_], F32, name=f"SQ{name}{h}",
                                    tag="SQ", bufs=2)
                    N2 = fpool.tile([HS, 1], F32, name=f"N2{name}{h}",
                                    tag="N2", bufs=2)
                    nc.scalar.activation(
                        SQ[:], EF[:], mybir.ActivationFunctionType.Square,
                        accum_out=N2[:],
                    )
                    NR = fpool.tile([HS, 1], F32, name=f"NR{name}{h}",
                                    tag="NR", bufs=2)
                    nc.scalar.activation(
                        NR[:], N2[:], mybir.ActivationFunctionType.Sqrt)
                    RN = fpool.tile([HS, 1], F32, name=f"RN{name}{h}",
                                    tag="RN", bufs=2)
                    nc.vector.reciprocal(RN[:], NR[:])
                    E = fpool.tile([HS, dp_], BF, name=f"E{name}{h}")
                    nc.vector.tensor_scalar_mul(E[:], EF[:], RN[:])
                    feats[(name, h)] = E

            STALL = [fpool.tile([128, PW], BF, name=f"STALL{h}")
                     for h in range(2)]

            def transposes(h):
                secs = [
                    (feats[("e5", h)], 0, 128, SEC["e5"]),
                    (feats[("e5", h)], 128, D5 - 128, SEC["e5lo"]),
                    (feats[("e2", h)], 0, D2, SEC["e2"]),
                    (feats[("e3", h)], 0, D3, SEC["e3"]),
                ]
                for si, (E, d0, dsz, col) in enumerate(secs):
                    pt = pp.tile([128, HS], BF, tag="ps2", bufs=3)
                    nc.tensor.transpose(
                        pt[:dsz, :], E[:, d0:d0 + dsz], IDB[:HS, :HS]
                    )
                    nc.any.tensor_copy(
                        STALL[h][:dsz, col:col + HS], pt[:dsz, :])

            def gather(h):
                nc.sync.dma_start(PKD[h][:], STALL[h][:])
                nc.gpsimd.collective_compute(
                    "AllGather",
                    mybir.AluOpType.bypass,
                    replica_groups=[list(range(N_CORES))],
                    ins=[PKD[h][:]],
                    outs=[GD[h][:]],
                )

            ph1(0)
            t2a0 = ph2mm(0)
            ph1(1)
            diagnorm(0, t2a0)
            transposes(0)
            gather(0)
            t2a1 = ph2mm(1)
            diagnorm(1, t2a1)
            transposes(1)
            gather(1)

        if DBG[0]:
            nc.sync.dma_start(io["o_gd0"][:], GD[0][:])
            nc.sync.dma_start(io["o_gd1"][:], GD[1][:])
            nc.sync.dma_start(io["o_pk0"][:], PKD[0][:])
            nc.sync.dma_start(io["o_pk1"][:], PKD[1][:])
        # ---------------- phase 3: tail (identical on all cores) ------------
        _tail(nc, tc, GD, o_dist)
        dpool_.release()
        apool.release()


def _tail(nc, tc, GD, o_dist):
    # b-sample rows: half0 -> global rows {0-12, 26-38, 52-63} (38 rows),
    #                half1 -> global rows {13-25, 39-51} (26 rows)
    with tc.tile_pool(name="tail", bufs=1) as tp, \
         tc.tile_pool(name="ps3", bufs=1, space="PSUM") as pp:
        chunks = [("e5", 128, SEC["e5"]), ("e5lo", D5 - 128, SEC["e5lo"]),
                  ("e2", D2, SEC["e2"]), ("e3", D3, SEC["e3"])]
        GDs = []
        FB = {}
        for h in range(2):
            g = tp.tile([128, N_CORES, PW], BF, name=f"GDs{h}")
            nc.sync.dma_start(
                g[:], GD[h][:].rearrange("(c r) f -> r c f", c=N_CORES))
            GDs.append(g)
        for nm, rows, col in chunks:
            FB0 = tp.tile([128, 38], BF, name=f"FB0{nm}", tag=f"FB0{nm}")
            nc.any.tensor_copy(
                FB0[:rows, 0:26].rearrange("r (c s) -> r c s", c=2),
                GDs[0][:rows, 0:2, col:col + 13],
            )
            nc.any.tensor_copy(FB0[:rows, 26:38],
                               GDs[0][:rows, 2, col:col + 12])
            FB1 = tp.tile([128, 26], BF, name=f"FB1{nm}", tag=f"FB1{nm}")
            nc.any.tensor_copy(
                FB1[:rows, 0:26].rearrange("r (c s) -> r c s", c=2),
                GDs[1][:rows, 0:2, col:col + 13],
            )
            FB[(nm, 0)] = FB0
            FB[(nm, 1)] = FB1

        # cosine dots; CS[hb] psum accumulates all 4 K-chunks x 4 q-blocks
        # q-col map: (c,h,sl) -> 26c+13h+sl-64 ; view [14:144] as (c5,s26):
        # col 14+26c+t with t=sl (h=0) or t=13+sl (h=1)
        CS = [pp.tile([38, SQN], F32, name="CS0", bufs=1),
              pp.tile([26, SQN], F32, name="CS1", bufs=1)]
        nb = [38, 26]
        qblocks = [(0, 1, 0, 2, 12, 13)]
        qblocks += [(14 + 26 * c, 27 + 26 * c, 0, 3 + c, 0, 13)
                    for c in range(5)]
        qblocks += [(1, 14, 1, 2, 0, 13)]
        qblocks += [(27 + 26 * c, 40 + 26 * c, 1, 3 + c, 0, 13)
                    for c in range(5)]
        for hb in range(2):
            for q0, q1, gh, gc, s0, s1 in qblocks:
                for ci_, (nm, rows, col) in enumerate(chunks):
                    nc.tensor.matmul(
                        CS[hb][:, q0:q1], lhsT=FB[(nm, hb)][:rows, :],
                        rhs=GDs[gh][:rows, gc, col + s0:col + s1],
                        start=(ci_ == 0), stop=(ci_ == 3))

        for hb in range(2):
            DIST = tp.tile([nb[hb], 9], F32, name=f"DIST{hb}", tag="DIST",
                           bufs=2)
            nc.vector.tensor_reduce(
                DIST[:], CS[hb][:].rearrange("p (s q) -> p s q", q=16),
                axis=mybir.AxisListType.X, op=mybir.AluOpType.add,
            )
            nc.scalar.activation(DIST[:], DIST[:],
                                 mybir.ActivationFunctionType.Copy,
                                 scale=1.0 / 48.0)
            # rows land in block order [c0h0, c1h0, c2h0, c0h1, c1h1];
            # the host un-permutes (see kernel())
            if hb == 0:
                nc.sync.dma_start(o_dist[0:38, :], DIST[0:38, :])
            else:
                nc.sync.dma_start(o_dist[38:64, :], DIST[0:26, :])


def _make_io(nc):
    io = {
        "xs": nc.dram_tensor("xs", [NS, CI, H], BF, kind="ExternalInput").ap(),
        "w1t": nc.dram_tensor("w1t", [128, 5, 2, 128], BF,
                              kind="ExternalInput").ap(),
        "w23t": nc.dram_tensor("w23t", [128, 2, 7], BF,
                               kind="ExternalInput").ap(),
        "cb": nc.dram_tensor("cb", [128, 2], F32, kind="ExternalInput").ap(),
        "o_dist": nc.dram_tensor("o_dist", [64, 9], F32,
                                 kind="ExternalOutput").ap(),
    }
    if DBG[0]:
        io["o_gd0"] = nc.dram_tensor("o_gd0", [128 * N_CORES, PW], BF,
                                     kind="ExternalOutput").ap()
        io["o_gd1"] = nc.dram_tensor("o_gd1", [128 * N_CORES, PW], BF,
                                     kind="ExternalOutput").ap()
        io["o_pk0"] = nc.dram_tensor("o_pk0", [128, PW], BF,
                                     kind="ExternalOutput").ap()
        io["o_pk1"] = nc.dram_tensor("o_pk1", [128, PW], BF,
                                     kind="ExternalOutput").ap()
    return io


def _build():
    nc = bacc.Bacc("TRN2", target_bir_lowering=False, debug=False,
                   num_devices=N_CORES)
    io = _make_io(nc)
    with tile.TileContext(nc) as tc:
        _emit(nc, tc, io)
    nc.compile()
    return nc


def _prep_inputs(input1, questions, labels, conv1_w, conv1_b, conv2_w, conv2_b,
                 conv2_2_w, conv2_2_b, sym_w, sym_b):
    bf = ml_dtypes.bfloat16
    xall = np.concatenate(
        [np.asarray(input1, np.float32),
         np.asarray(questions, np.float32).reshape(SQN, CI, H)], axis=0
    ).astype(bf)
    w1t = np.ascontiguousarray(
        np.asarray(conv1_w, np.float32).reshape(2, 128, CI, 5)
        .transpose(2, 3, 0, 1)
    ).astype(bf)  # [ci, k, cc, co]
    w2t = np.asarray(conv2_w, np.float32)[0].reshape(2, 128, 5) \
        .transpose(1, 0, 2)
    w22t = np.asarray(conv2_2_w, np.float32)[0].reshape(2, 128, 2) \
        .transpose(1, 0, 2)
    w23t = np.ascontiguousarray(
        np.concatenate([w2t, w22t], axis=2)).astype(bf)  # [ci, cc, 7]
    c5 = float((conv2_w[0].sum(axis=1) * conv1_b).sum() + conv2_b[0])
    c2 = float((conv2_2_w[0].sum(axis=1) * conv1_b).sum() + conv2_2_b[0])
    cb = np.stack([np.full(128, c5, np.float32),
                   np.full(128, c2, np.float32)], axis=1)
    common = {"w1t": w1t, "w23t": w23t, "cb": cb}
    in_maps = []
    for c in range(N_CORES):
        m = dict(common)
        m["xs"] = np.ascontiguousarray(xall[c * NS:(c + 1) * NS])
        in_maps.append(m)
    return in_maps


def kernel(input1, questions, labels, conv1_w, conv1_b, conv2_w, conv2_b,
           conv2_2_w, conv2_2_b, sym_w, sym_b, _trace=False):
    if "nc" not in _cached:
        _cached["nc"] = _build()
    nc = _cached["nc"]
    in_maps = _prep_inputs(input1, questions, labels, conv1_w, conv1_b,
                           conv2_w, conv2_b, conv2_2_w, conv2_2_b,
                           sym_w, sym_b)
    res = bass_utils.run_bass_kernel_spmd(
        nc, in_maps, core_ids=list(range(N_CORES)), trace=_trace,
    )
    _cached["last_results"] = res
    r0 = res.results[0]
    dperm = r0["o_dist"]
    # device rows are [c0h0(13), c1h0(13), c2h0(12), c0h1(13), c1h1(13)]
    order = (list(range(0, 13)) + list(range(26, 39)) + list(range(52, 64))
             + list(range(13, 26)) + list(range(39, 52)))
    dist = np.empty_like(dperm)
    dist[order] = dperm
    # tiny tail (≈1e3 flops), same formulas as the model
    logits = dist.astype(np.float64) @ np.asarray(sym_w, np.float64).T \
        + np.asarray(sym_b, np.float64)
    m = logits.max(axis=1, keepdims=True)
    e = np.exp(logits - m)
    out = e / e.sum(axis=1, keepdims=True)
    lse = np.log(np.exp(out).sum(axis=1, keepdims=True))
    logp = out - lse
    lab = np.asarray(labels).astype(np.int64)
    loss = -np.mean(logp[np.arange(out.shape[0]), lab])
    return (np.float32(loss), out.astype(np.float32), dist)
